# revision 7
# baseline (speedup 1.0000x reference)
"""GRU-cell-variant kernel for Trainium2, data-parallel over batch on 8 cores.

Reference (per batch row b, hidden size H=1024):
    gates = sigmoid(x @ W_ih + b_ih + h @ W_hh + b_hh)   # [B, 2H]
    z, r  = gates[:, :H], gates[:, H:]
    cand  = tanh(x @ W_c + b_c + r * (h @ W_hc + b_hc))
    out   = (1 - z) * h + z * cand

Design:
  - 8-way batch shard (1024 rows/core), weights replicated. No collectives.
  - Everything on-chip is computed TRANSPOSED: out.T[o, b].
  - Mixed-precision matmuls chosen by measured error contribution to the
    final output (numpy sim of quantization, rel-err budget 2e-2):
      * z-gate, r-gate, h@W_hc: fp8e4 double-pumped (DoubleRow perf mode,
        2 contraction k-tiles per pass = 2x PE throughput), weights
        pre-scaled by 128 so uniform(-1/32,1/32) entries clear the e4m3
        subnormal floor; the 1/128 is folded into ACT activation scales.
      * x@W_c: moving operand in fp8e3 (E3M4, x fits its [~-5.2,5.2]
        range directly), stationary W_c in fp16 x128 — full-rate matmul,
        half the activation bytes of fp16, W-side error ~zero.
    Simulated rel err 1.55e-2 (sim matches HW to ~1e-6).
  - fp32 PSUM accumulation; elementwise fp32; h-residual fp16; out fp16.
  - Scheduling: weight loads for tile j+1 issue at the top of tile j so
    they queue ahead of j's output DMAs (engine queues are in-order);
    outputs ride the sync ring, weights/acts split across both rings;
    cold start streams both gates' x-pair matmuls before the h-pairs to
    match DMA arrival; the final block's blend runs in two 256-wide
    halves so the tail chain pipelines.
"""

import numpy as np
import ml_dtypes

import concourse.bass as bass
import concourse.mybir as mybir
import concourse.tile as tile
from concourse import bacc
from concourse.bass_utils import run_bass_kernel_spmd

N_CORES = 8
B = 8192
H = 1024
BL = B // N_CORES  # batch rows per core
P = 128
KC = H // P  # 8 contraction chunks of 128 per 1024-wide operand
NJ = H // P  # 8 hidden-dim tiles
NB = BL // 512  # 2 moving halves of 512 batch columns

WS = 128.0  # host-side weight pre-scale (power of two, exact)

F8 = mybir.dt.float8e4
E3 = mybir.dt.float8e3
F16 = mybir.dt.float16
F32 = mybir.dt.float32
AF = mybir.ActivationFunctionType
ALU = mybir.AluOpType
DR = mybir.MatmulPerfMode.DoubleRow

_CACHE = {}


def _build_program():
    nc = bacc.Bacc(
        "TRN2",
        target_bir_lowering=False,
        debug=False,
        enable_asserts=False,
        num_devices=N_CORES,
    )

    # DRAM inputs, packed on the host into SBUF-friendly layouts.
    # x8/h8:  [p, kc*BL + b]       = x[b, kc*128 + p]            (fp8e4)
    # xe3:    same layout, fp8e3 (candidate x@W_c moving operand)
    # h16:    [p, j*BL + b] fp16 (residual path)
    # Wg:     [p, t*2048 + kc*128 + jj] = 128*Wg_full[kc*128+p, t*128+jj] (fp8e4)
    # Wc:     [p, j*1024 + kc*128 + jj] = 128*W_c[kc*128+p, j*128+jj] (fp16)
    # Whc:    same layout as Wc, fp8e4, x128
    # bg:     [p, t] = (b_ih+b_hh)[t*128+p] unscaled; bc unscaled; bhc x128
    x8 = nc.dram_tensor("x8", [P, KC * BL], F8, kind="ExternalInput").ap()
    h8 = nc.dram_tensor("h8", [P, KC * BL], F8, kind="ExternalInput").ap()
    xe3 = nc.dram_tensor("xe3", [P, KC * BL], E3, kind="ExternalInput").ap()
    h16 = nc.dram_tensor("h16", [P, NJ * BL], F16, kind="ExternalInput").ap()
    Wg = nc.dram_tensor("Wg", [P, 16 * 2048], F8, kind="ExternalInput").ap()
    Wc = nc.dram_tensor("Wc", [P, NJ * H], F16, kind="ExternalInput").ap()
    Whc = nc.dram_tensor("Whc", [P, NJ * H], F8, kind="ExternalInput").ap()
    bg = nc.dram_tensor("bg", [P, 16], F32, kind="ExternalInput").ap()
    bc = nc.dram_tensor("bc", [P, NJ], F32, kind="ExternalInput").ap()
    bhc = nc.dram_tensor("bhc", [P, NJ], F32, kind="ExternalInput").ap()
    outT = nc.dram_tensor("outT", [P, NJ * BL], F16, kind="ExternalOutput").ap()

    inv = 1.0 / WS

    with tile.TileContext(nc) as tc:
        with (
            tc.tile_pool(name="const", bufs=1) as cpool,
            tc.tile_pool(name="wg", bufs=4) as wgpool,
            tc.tile_pool(name="wsm", bufs=4) as wsmpool,
            tc.tile_pool(name="psum", bufs=8, space="PSUM") as ppool,
            tc.tile_pool(name="gates", bufs=8) as gpool,
            tc.tile_pool(name="work", bufs=12) as wpool,
        ):
            bg_sb = cpool.tile([P, 16], F32, tag="bg")
            bc_sb = cpool.tile([P, NJ], F32, tag="bc")
            bhc_sb = cpool.tile([P, NJ], F32, tag="bhc")

            x8_sb = cpool.tile([P, KC * BL], F8, tag="x8")
            h8_sb = cpool.tile([P, KC * BL], F8, tag="h8")
            xe3_sb = cpool.tile([P, KC * BL], E3, tag="xe3")
            h16_sb = cpool.tile([P, NJ * BL], F16, tag="h16")

            # 3D views [p, kc, *] for chunked DMA + DoubleRow k-pair slices
            xs8 = x8_sb[:].rearrange("p (kc b) -> p kc b", kc=KC)
            xd8 = x8.rearrange("p (kc b) -> p kc b", kc=KC)
            hs8 = h8_sb[:].rearrange("p (kc b) -> p kc b", kc=KC)
            hd8 = h8.rearrange("p (kc b) -> p kc b", kc=KC)
            xs3 = xe3_sb[:].rearrange("p (kc b) -> p kc b", kc=KC)
            xd3 = xe3.rearrange("p (kc b) -> p kc b", kc=KC)

            def gate_pairs(psum, w3, b0, qs, start, stop):
                # pairs 0-3 read x8, pairs 4-7 read h8 (16 k-tiles over [x;h])
                for i, q in enumerate(qs):
                    src = xs8 if q < 4 else hs8
                    kk = (q % 4) * 2
                    nc.tensor.matmul(
                        psum[:],
                        lhsT=w3[:, 2 * q : 2 * q + 2, :],
                        rhs=src[:, kk : kk + 2, b0 : b0 + 512],
                        start=(start and i == 0),
                        stop=(stop and i == len(qs) - 1),
                        perf_mode=DR,
                    )

            def gate_dr(psum, w3, b0):
                gate_pairs(psum, w3, b0, range(8), True, True)

            def hc_dr(psum, w3, b0):
                for q in range(4):
                    nc.tensor.matmul(
                        psum[:],
                        lhsT=w3[:, 2 * q : 2 * q + 2, :],
                        rhs=hs8[:, 2 * q : 2 * q + 2, b0 : b0 + 512],
                        start=(q == 0),
                        stop=(q == 3),
                        perf_mode=DR,
                    )

            def xc_mm(psum, w_sb, b0):
                for kc in range(KC):
                    off = kc * BL + b0
                    nc.tensor.matmul(
                        psum[:],
                        lhsT=w_sb[:, kc * P : (kc + 1) * P],
                        rhs=xe3_sb[:, off : off + 512],
                        start=(kc == 0),
                        stop=(kc == KC - 1),
                    )

            def alloc_w():
                wz = wgpool.tile([P, 2048], F8, tag="wg")
                wr = wgpool.tile([P, 2048], F8, tag="wg")
                whc_w = wsmpool.tile([P, H], F8, tag="whc")
                wc_w = wsmpool.tile([P, H], F16, tag="wc")
                return (
                    wz, wr, whc_w, wc_w,
                    wz[:].rearrange("p (kc m) -> p kc m", kc=16),
                    wr[:].rearrange("p (kc m) -> p kc m", kc=16),
                    whc_w[:].rearrange("p (kc m) -> p kc m", kc=KC),
                )

            def load_w(tiles, j):
                wz, wr, whc_w, wc_w = tiles[:4]
                nc.sync.dma_start(wz[:], Wg[:, j * 2048 : (j + 1) * 2048])
                nc.sync.dma_start(whc_w[:], Whc[:, j * H : (j + 1) * H])
                nc.scalar.dma_start(wr[:], Wg[:, (NJ + j) * 2048 : (NJ + j + 1) * 2048])
                nc.scalar.dma_start(wc_w[:], Wc[:, j * H : (j + 1) * H])
                nc.scalar.dma_start(
                    h16_sb[:, j * BL : (j + 1) * BL], h16[:, j * BL : (j + 1) * BL]
                )

            def sigmoid_z(pz, j):
                z_sb = gpool.tile([P, 512], F32, tag="g")
                nc.scalar.activation(
                    z_sb[:], pz[:], AF.Sigmoid, bias=bg_sb[:, j : j + 1], scale=inv
                )
                return z_sb

            def sigmoid_r(pr, j):
                r_sb = gpool.tile([P, 512], F32, tag="g")
                nc.scalar.activation(
                    r_sb[:], pr[:], AF.Sigmoid,
                    bias=bg_sb[:, NJ + j : NJ + j + 1], scale=inv,
                )
                return r_sb

            def make_zh(z_sb, hoff):
                # zh = (z - 1) * h, off the critical path
                zh = wpool.tile([P, 512], F32, tag="w")
                nc.vector.scalar_tensor_tensor(
                    zh[:], z_sb[:], 1.0, h16_sb[:, hoff : hoff + 512],
                    ALU.subtract, ALU.mult,
                )
                return zh

            def blend_head(j, ph, px, r_sb, halves=1):
                # rh = (hc*128 + bhc*128) * r ; s = px + rh   (x128 scale)
                wd = 512 // halves
                rh = wpool.tile([P, 512], F32, tag="w")
                s = wpool.tile([P, 512], F32, tag="w")
                for hv in range(halves):
                    sl = slice(hv * wd, (hv + 1) * wd)
                    nc.vector.scalar_tensor_tensor(
                        rh[:, sl], ph[:, sl], bhc_sb[:, j : j + 1], r_sb[:, sl],
                        ALU.add, ALU.mult,
                    )
                    nc.vector.tensor_add(s[:, sl], px[:, sl], rh[:, sl])
                return s

            def blend_tail(j, hoff, s, z_sb, zh, halves=1):
                wd = 512 // halves
                cand = wpool.tile([P, 512], F32, tag="w")
                m = wpool.tile([P, 512], F32, tag="w")
                o_sb = wpool.tile([P, 512], F16, tag="o")
                for hv in range(halves):
                    lo = hv * wd
                    sl = slice(lo, lo + wd)
                    nc.scalar.activation(
                        cand[:, sl], s[:, sl], AF.Tanh,
                        bias=bc_sb[:, j : j + 1], scale=inv,
                    )
                    # out = z*cand - (z-1)*h
                    nc.vector.tensor_mul(m[:, sl], z_sb[:, sl], cand[:, sl])
                    nc.vector.tensor_sub(o_sb[:, sl], m[:, sl], zh[:, sl])
                    nc.sync.dma_start(outT[:, hoff + lo : hoff + lo + wd], o_sb[:, sl])

            def blend(j, hoff, ph, px, r_sb, z_sb, zh, halves=1):
                s = blend_head(j, ph, px, r_sb, halves)
                blend_tail(j, hoff, s, z_sb, zh, halves)

            # ---------------- j = 0: cold start ----------------
            cur = alloc_w()
            nxt = alloc_w()
            wz0, wr0, whc0, wc0, wz03, wr03, whc03 = cur

            # sync ring: the fp8 gate activations in consumption order.
            nc.sync.dma_start(xs8[:, 0:4, 0:512], xd8[:, 0:4, 0:512])
            nc.sync.dma_start(xs8[:, 4:8, 0:512], xd8[:, 4:8, 0:512])
            nc.sync.dma_start(hs8[:, 0:4, 0:512], hd8[:, 0:4, 0:512])
            nc.sync.dma_start(hs8[:, 4:8, 0:512], hd8[:, 4:8, 0:512])
            nc.sync.dma_start(xs8[:, :, 512:1024], xd8[:, :, 512:1024])
            nc.sync.dma_start(hs8[:, :, 512:1024], hd8[:, :, 512:1024])
            nc.sync.dma_start(xs3[:, :, 512:1024], xd3[:, :, 512:1024])

            # scalar ring: gate weights first (these become static DMAs and
            # feed the very first matmuls), then the candidate feeds, then
            # the rest of j0's consts and j1's prefetch.
            t0 = NJ * 2048
            nc.scalar.dma_start(wz0[:, 0:1024], Wg[:, 0:1024])
            nc.scalar.dma_start(wz0[:, 1024:2048], Wg[:, 1024:2048])
            nc.scalar.dma_start(wr0[:, 0:1024], Wg[:, t0 : t0 + 1024])
            nc.scalar.dma_start(wr0[:, 1024:2048], Wg[:, t0 + 1024 : t0 + 2048])
            nc.scalar.dma_start(bg_sb[:], bg[:])
            nc.scalar.dma_start(whc0[:], Whc[:, 0:H])
            nc.scalar.dma_start(xs3[:, :, 0:512], xd3[:, :, 0:512])
            nc.scalar.dma_start(wc0[:], Wc[:, 0:H])
            nc.scalar.dma_start(h16_sb[:, 0:BL], h16[:, 0:BL])
            nc.scalar.dma_start(bc_sb[:], bc[:])
            nc.scalar.dma_start(bhc_sb[:], bhc[:])
            # j1 prefetch, all on the scalar ring (sync still streams b1 acts)
            nc.scalar.dma_start(nxt[0][:], Wg[:, 1 * 2048 : 2 * 2048])
            nc.scalar.dma_start(nxt[2][:], Whc[:, 1 * H : 2 * H])
            nc.scalar.dma_start(nxt[1][:], Wg[:, (NJ + 1) * 2048 : (NJ + 2) * 2048])
            nc.scalar.dma_start(nxt[3][:], Wc[:, H : 2 * H])
            nc.scalar.dma_start(h16_sb[:, BL : 2 * BL], h16[:, BL : 2 * BL])

            # compute j0 in DMA-arrival order: both gates' x-pairs first,
            # interleaved PSUM groups; px0 before the b1 h-pairs (its feed
            # rides the scalar ring and lands before h8's b1 half).
            pz0 = ppool.tile([P, 512], F32, tag="ps")
            pr0 = ppool.tile([P, 512], F32, tag="ps")
            gate_pairs(pz0, wz03, 0, range(4), True, False)
            gate_pairs(pr0, wr03, 0, range(4), True, False)
            gate_pairs(pz0, wz03, 0, range(4, 8), False, True)
            gate_pairs(pr0, wr03, 0, range(4, 8), False, True)
            z0 = sigmoid_z(pz0, 0)
            zh0 = make_zh(z0, 0)
            r0 = sigmoid_r(pr0, 0)
            ph0 = ppool.tile([P, 512], F32, tag="ps")
            hc_dr(ph0, whc03, 0)

            pz1 = ppool.tile([P, 512], F32, tag="ps")
            pr1 = ppool.tile([P, 512], F32, tag="ps")
            gate_pairs(pz1, wz03, 512, range(4), True, False)
            gate_pairs(pr1, wr03, 512, range(4), True, False)
            px0 = ppool.tile([P, 512], F32, tag="ps")
            xc_mm(px0, wc0, 0)
            gate_pairs(pz1, wz03, 512, range(4, 8), False, True)
            gate_pairs(pr1, wr03, 512, range(4, 8), False, True)
            z1 = sigmoid_z(pz1, 0)
            zh1 = make_zh(z1, 512)
            r1 = sigmoid_r(pr1, 0)
            blend(0, 0, ph0, px0, r0, z0, zh0)

            ph1 = ppool.tile([P, 512], F32, tag="ps")
            hc_dr(ph1, whc03, 512)
            px1 = ppool.tile([P, 512], F32, tag="ps")
            xc_mm(px1, wc0, 512)
            # The blend is software-pipelined one block behind the matmuls:
            # rh/s (head) emit right after the block's px so they run the
            # moment the psums land, while tanh/mul/sub (tail) emit at the
            # start of the NEXT block — the tanh then fires immediately at
            # the window boundary instead of making the DVE queue idle-wait
            # mid-window, so the DVE/ACT chain never falls behind the PE.
            s1v = blend_head(0, ph1, px1, r1)
            pending = (0, 512, s1v, z1, zh1, 1)

            cur = nxt

            # ---------------- j = 1..7: steady state ----------------
            for j in range(1, NJ):
                if j + 1 < NJ:
                    nxt = alloc_w()
                    load_w(nxt, j + 1)
                wz, wr, whc_w, wc_w, wz3, wr3, whc3 = cur

                for b in range(NB):
                    b0 = b * 512
                    hoff = j * BL + b0
                    last = j == NJ - 1 and b == NB - 1

                    blend_tail(*pending)
                    pz = ppool.tile([P, 512], F32, tag="ps")
                    gate_dr(pz, wz3, b0)
                    z_sb = sigmoid_z(pz, j)
                    zh = make_zh(z_sb, hoff)
                    pr = ppool.tile([P, 512], F32, tag="ps")
                    gate_dr(pr, wr3, b0)
                    r_sb = sigmoid_r(pr, j)

                    ph = ppool.tile([P, 512], F32, tag="ps")
                    hc_dr(ph, whc3, b0)
                    px = ppool.tile([P, 512], F32, tag="ps")
                    xc_mm(px, wc_w, b0)
                    s_v = blend_head(j, ph, px, r_sb, halves=2 if last else 1)
                    pending = (j, hoff, s_v, z_sb, zh, 2 if last else 1)

                cur = nxt

            blend_tail(*pending)

    nc.compile()
    return nc


def _pack_weights(W_ih, b_ih, W_hh, b_hh, W_c, b_c, W_hc, b_hc):
    f8 = ml_dtypes.float8_e4m3
    Wg_full = np.concatenate([W_ih, W_hh], axis=0)  # [2H, 2H] = [k, o]
    WgH = np.ascontiguousarray(
        (Wg_full * WS).reshape(16, P, 16, P).transpose(1, 2, 0, 3).reshape(P, 16 * 2048)
    ).astype(f8)
    WcH = np.ascontiguousarray(
        (W_c * WS).reshape(KC, P, NJ, P).transpose(1, 2, 0, 3).reshape(P, NJ * H)
    ).astype(np.float16)
    WhcH = np.ascontiguousarray(
        (W_hc * WS).reshape(KC, P, NJ, P).transpose(1, 2, 0, 3).reshape(P, NJ * H)
    ).astype(f8)
    bgH = np.ascontiguousarray((b_ih + b_hh).reshape(16, P).T).astype(np.float32)
    bcH = np.ascontiguousarray(b_c.reshape(NJ, P).T).astype(np.float32)
    bhcH = np.ascontiguousarray((b_hc * WS).reshape(NJ, P).T).astype(np.float32)
    return WgH, WcH, WhcH, bgH, bcH, bhcH


def _pack_acts(a, dtype):
    # [BL, H] -> [p, kc*BL + b] with a[b, kc*128+p]
    return np.ascontiguousarray(
        a.T.reshape(KC, P, BL).transpose(1, 0, 2).reshape(P, KC * BL)
    ).astype(dtype)


def _make_in_maps(input, hx, W_ih, b_ih, W_hh, b_hh, W_c, b_c, W_hc, b_hc):
    input = np.asarray(input, np.float32)
    hx = np.asarray(hx, np.float32)
    WgH, WcH, WhcH, bgH, bcH, bhcH = _pack_weights(
        np.asarray(W_ih, np.float32), np.asarray(b_ih, np.float32),
        np.asarray(W_hh, np.float32), np.asarray(b_hh, np.float32),
        np.asarray(W_c, np.float32), np.asarray(b_c, np.float32),
        np.asarray(W_hc, np.float32), np.asarray(b_hc, np.float32),
    )
    f8 = ml_dtypes.float8_e4m3
    e3 = ml_dtypes.float8_e3m4
    in_maps = []
    for i in range(N_CORES):
        xs = input[i * BL : (i + 1) * BL]
        hs = hx[i * BL : (i + 1) * BL]
        in_maps.append(
            {
                "x8": _pack_acts(xs, f8),
                "h8": _pack_acts(hs, f8),
                "xe3": _pack_acts(xs, e3),
                "h16": _pack_acts(hs, np.float16),
                "Wg": WgH,
                "Wc": WcH,
                "Whc": WhcH,
                "bg": bgH,
                "bc": bcH,
                "bhc": bhcH,
            }
        )
    return in_maps


def kernel(input, hx, W_ih, b_ih, W_hh, b_hh, W_c, b_c, W_hc, b_hc):
    if "nc" not in _CACHE:
        _CACHE["nc"] = _build_program()
    nc = _CACHE["nc"]

    in_maps = _make_in_maps(
        input, hx, W_ih, b_ih, W_hh, b_hh, W_c, b_c, W_hc, b_hc
    )

    res = run_bass_kernel_spmd(nc, in_maps, core_ids=list(range(N_CORES)))
    out = np.empty((B, H), np.float32)
    for i, r in enumerate(res.results):
        o = np.asarray(r["outT"], np.float32).reshape(P, NJ, BL).transpose(2, 1, 0).reshape(BL, H)
        out[i * BL : (i + 1) * BL] = o
    return out


# revision 11
# speedup vs baseline: 1.2405x; 1.2405x over previous
"""GRU-cell-variant kernel for Trainium2, data-parallel over batch on 8 cores.

Reference (per batch row b, hidden size H=1024):
    gates = sigmoid(x @ W_ih + b_ih + h @ W_hh + b_hh)   # [B, 2H]
    z, r  = gates[:, :H], gates[:, H:]
    cand  = tanh(x @ W_c + b_c + r * (h @ W_hc + b_hc))
    out   = (1 - z) * h + z * cand

Design:
  - 8-way batch shard (1024 rows/core), weights replicated. No collectives.
  - Everything on-chip is computed TRANSPOSED: out.T[o, b].
  - Mixed-precision matmuls chosen by measured error contribution to the
    final output (numpy sim of quantization, rel-err budget 2e-2):
      * z-gate, r-gate, h@W_hc: fp8e4 double-pumped (DoubleRow perf mode,
        2 contraction k-tiles per pass = 2x PE throughput), weights
        pre-scaled by 128 so uniform(-1/32,1/32) entries clear the e4m3
        subnormal floor; the 1/128 is folded into ACT activation scales.
      * x@W_c: moving operand in fp8e3 (E3M4, x fits its [~-5.2,5.2]
        range directly), stationary W_c in fp16 x128 — full-rate matmul,
        half the activation bytes of fp16, W-side error ~zero.
    Simulated rel err 1.55e-2 (sim matches HW to ~1e-6).
  - fp32 PSUM accumulation; elementwise fp32; h-residual fp16; out fp16.
  - Scheduling: weight loads for tile j+1 issue at the top of tile j so
    they queue ahead of j's output DMAs (engine queues are in-order);
    outputs ride the sync ring, weights/acts split across both rings;
    cold start streams both gates' x-pair matmuls before the h-pairs to
    match DMA arrival; the final block's blend runs in two 256-wide
    halves so the tail chain pipelines.
"""

import numpy as np
import ml_dtypes

import concourse.bass as bass
import concourse.mybir as mybir
import concourse.tile as tile
from concourse import bacc
from concourse.bass_utils import run_bass_kernel_spmd

N_CORES = 8
B = 8192
H = 1024
BL = B // N_CORES  # batch rows per core
P = 128
KC = H // P  # 8 contraction chunks of 128 per 1024-wide operand
NJ = H // P  # 8 hidden-dim tiles
NB = BL // 512  # 2 moving halves of 512 batch columns

WS = 128.0  # host-side weight pre-scale (power of two, exact)

F8 = mybir.dt.float8e4
E3 = mybir.dt.float8e3
F16 = mybir.dt.float16
F32 = mybir.dt.float32
AF = mybir.ActivationFunctionType
ALU = mybir.AluOpType
DR = mybir.MatmulPerfMode.DoubleRow

_CACHE = {}


def _build_program():
    nc = bacc.Bacc(
        "TRN2",
        target_bir_lowering=False,
        debug=False,
        enable_asserts=False,
        num_devices=N_CORES,
    )

    # DRAM inputs, packed on the host into SBUF-friendly layouts.
    # x8/h8:  [p, kc*BL + b]       = x[b, kc*128 + p]            (fp8e4)
    # xe3:    same layout, fp8e3 (candidate x@W_c moving operand)
    # h16:    [p, j*BL + b] fp16 (residual path)
    # Wg:     [p, t*2048 + kc*128 + jj] = 128*Wg_full[kc*128+p, t*128+jj] (fp8e4)
    # Wc:     [p, j*1024 + kc*128 + jj] = 128*W_c[kc*128+p, j*128+jj] (fp16)
    # Whc:    same layout as Wc, fp8e4, x128
    # bg:     [p, t] = (b_ih+b_hh)[t*128+p] unscaled; bc unscaled; bhc x128
    x8 = nc.dram_tensor("x8", [P, KC * BL], F8, kind="ExternalInput").ap()
    h8 = nc.dram_tensor("h8", [P, KC * BL], F8, kind="ExternalInput").ap()
    xe3 = nc.dram_tensor("xe3", [P, KC * BL], E3, kind="ExternalInput").ap()
    h16 = nc.dram_tensor("h16", [P, NJ * BL], F16, kind="ExternalInput").ap()
    Wg = nc.dram_tensor("Wg", [P, 16 * 2048], F8, kind="ExternalInput").ap()
    Wc = nc.dram_tensor("Wc", [P, NJ * H], F16, kind="ExternalInput").ap()
    Whc = nc.dram_tensor("Whc", [P, NJ * H], F8, kind="ExternalInput").ap()
    bg = nc.dram_tensor("bg", [P, 16], F32, kind="ExternalInput").ap()
    bc = nc.dram_tensor("bc", [P, NJ], F32, kind="ExternalInput").ap()
    bhc = nc.dram_tensor("bhc", [P, NJ], F32, kind="ExternalInput").ap()
    outT = nc.dram_tensor("outT", [P, NJ * BL], F16, kind="ExternalOutput").ap()

    inv = 1.0 / WS

    with tile.TileContext(nc) as tc:
        with (
            tc.tile_pool(name="const", bufs=1) as cpool,
            tc.tile_pool(name="wg", bufs=4) as wgpool,
            tc.tile_pool(name="wsm", bufs=4) as wsmpool,
            tc.tile_pool(name="psum", bufs=8, space="PSUM") as ppool,
            tc.tile_pool(name="gates", bufs=8) as gpool,
            tc.tile_pool(name="work", bufs=12) as wpool,
        ):
            bg_sb = cpool.tile([P, 16], F32, tag="bg")
            bc_sb = cpool.tile([P, NJ], F32, tag="bc")
            bhc_sb = cpool.tile([P, NJ], F32, tag="bhc")

            x8_sb = cpool.tile([P, KC * BL], F8, tag="x8")
            h8_sb = cpool.tile([P, KC * BL], F8, tag="h8")
            xe3_sb = cpool.tile([P, KC * BL], E3, tag="xe3")
            h16_sb = cpool.tile([P, NJ * BL], F16, tag="h16")

            # 3D views [p, kc, *] for chunked DMA + DoubleRow k-pair slices
            xs8 = x8_sb[:].rearrange("p (kc b) -> p kc b", kc=KC)
            xd8 = x8.rearrange("p (kc b) -> p kc b", kc=KC)
            hs8 = h8_sb[:].rearrange("p (kc b) -> p kc b", kc=KC)
            hd8 = h8.rearrange("p (kc b) -> p kc b", kc=KC)
            xs3 = xe3_sb[:].rearrange("p (kc b) -> p kc b", kc=KC)
            xd3 = xe3.rearrange("p (kc b) -> p kc b", kc=KC)

            def gate_pairs(psum, w3, b0, qs, start, stop):
                # pairs 0-3 read x8, pairs 4-7 read h8 (16 k-tiles over [x;h])
                for i, q in enumerate(qs):
                    src = xs8 if q < 4 else hs8
                    kk = (q % 4) * 2
                    nc.tensor.matmul(
                        psum[:],
                        lhsT=w3[:, 2 * q : 2 * q + 2, :],
                        rhs=src[:, kk : kk + 2, b0 : b0 + 512],
                        start=(start and i == 0),
                        stop=(stop and i == len(qs) - 1),
                        perf_mode=DR,
                    )

            def gate_dr(psum, w3, b0):
                gate_pairs(psum, w3, b0, range(8), True, True)

            def hc_dr(psum, w3, b0):
                for q in range(4):
                    nc.tensor.matmul(
                        psum[:],
                        lhsT=w3[:, 2 * q : 2 * q + 2, :],
                        rhs=hs8[:, 2 * q : 2 * q + 2, b0 : b0 + 512],
                        start=(q == 0),
                        stop=(q == 3),
                        perf_mode=DR,
                    )

            def xc_mm(psum, w_sb, b0):
                for kc in range(KC):
                    off = kc * BL + b0
                    nc.tensor.matmul(
                        psum[:],
                        lhsT=w_sb[:, kc * P : (kc + 1) * P],
                        rhs=xe3_sb[:, off : off + 512],
                        start=(kc == 0),
                        stop=(kc == KC - 1),
                    )

            def alloc_w():
                wz = wgpool.tile([P, 2048], F8, tag="wg")
                wr = wgpool.tile([P, 2048], F8, tag="wg")
                whc_w = wsmpool.tile([P, H], F8, tag="whc")
                wc_w = wsmpool.tile([P, H], F16, tag="wc")
                return (
                    wz, wr, whc_w, wc_w,
                    wz[:].rearrange("p (kc m) -> p kc m", kc=16),
                    wr[:].rearrange("p (kc m) -> p kc m", kc=16),
                    whc_w[:].rearrange("p (kc m) -> p kc m", kc=KC),
                )

            def load_w(tiles, j):
                wz, wr, whc_w, wc_w = tiles[:4]
                nc.sync.dma_start(wz[:], Wg[:, j * 2048 : (j + 1) * 2048])
                nc.sync.dma_start(whc_w[:], Whc[:, j * H : (j + 1) * H])
                nc.scalar.dma_start(wr[:], Wg[:, (NJ + j) * 2048 : (NJ + j + 1) * 2048])
                nc.scalar.dma_start(wc_w[:], Wc[:, j * H : (j + 1) * H])
                nc.scalar.dma_start(
                    h16_sb[:, j * BL : (j + 1) * BL], h16[:, j * BL : (j + 1) * BL]
                )

            def sigmoid_z(pz, j):
                z_sb = gpool.tile([P, 512], F32, tag="g")
                nc.scalar.activation(
                    z_sb[:], pz[:], AF.Sigmoid, bias=bg_sb[:, j : j + 1], scale=inv
                )
                return z_sb

            def sigmoid_r(pr, j):
                r_sb = gpool.tile([P, 512], F32, tag="g")
                nc.scalar.activation(
                    r_sb[:], pr[:], AF.Sigmoid,
                    bias=bg_sb[:, NJ + j : NJ + j + 1], scale=inv,
                )
                return r_sb

            def make_zh(z_sb, hoff):
                # v = (1 - z) * h = h - z*h, off the critical path. Runs on
                # the otherwise-idle GPSIMD engine (which supports plain
                # tensor_tensor but not scalar_tensor_tensor): keeping it off
                # the DVE queue gives the in-order DVE ~1.5us/block of slack
                # so the blend chain drains instead of accumulating a
                # multi-block backlog that serializes after the final matmul.
                hsl = h16_sb[:, hoff : hoff + 512]
                u = wpool.tile([P, 512], F32, tag="w")
                nc.gpsimd.tensor_mul(u[:], z_sb[:], hsl)
                v = wpool.tile([P, 512], F32, tag="w")
                nc.gpsimd.tensor_sub(v[:], hsl, u[:])
                return v

            def blend_head(j, ph, px, r_sb, halves=1):
                # rh = (hc*128 + bhc*128) * r ; s = px + rh   (x128 scale)
                wd = 512 // halves
                rh = wpool.tile([P, 512], F32, tag="w")
                s = wpool.tile([P, 512], F32, tag="w")
                for hv in range(halves):
                    sl = slice(hv * wd, (hv + 1) * wd)
                    nc.vector.scalar_tensor_tensor(
                        rh[:, sl], ph[:, sl], bhc_sb[:, j : j + 1], r_sb[:, sl],
                        ALU.add, ALU.mult,
                    )
                    nc.vector.tensor_add(s[:, sl], px[:, sl], rh[:, sl])
                return s

            def blend_tail(j, hoff, s, z_sb, zh, halves=1):
                wd = 512 // halves
                cand = wpool.tile([P, 512], F32, tag="w")
                m = wpool.tile([P, 512], F32, tag="w")
                o_sb = wpool.tile([P, 512], F16, tag="o")
                for hv in range(halves):
                    lo = hv * wd
                    sl = slice(lo, lo + wd)
                    nc.scalar.activation(
                        cand[:, sl], s[:, sl], AF.Tanh,
                        bias=bc_sb[:, j : j + 1], scale=inv,
                    )
                    # out = z*cand + (1-z)*h
                    nc.vector.tensor_mul(m[:, sl], z_sb[:, sl], cand[:, sl])
                    nc.vector.tensor_add(o_sb[:, sl], m[:, sl], zh[:, sl])
                    nc.sync.dma_start(outT[:, hoff + lo : hoff + lo + wd], o_sb[:, sl])

            def blend(j, hoff, ph, px, r_sb, z_sb, zh, halves=1):
                s = blend_head(j, ph, px, r_sb, halves)
                blend_tail(j, hoff, s, z_sb, zh, halves)

            # ---------------- j = 0: cold start ----------------
            cur = alloc_w()
            nxt = alloc_w()
            wz0, wr0, whc0, wc0, wz03, wr03, whc03 = cur

            # sync ring: the fp8 gate activations in consumption order.
            nc.sync.dma_start(xs8[:, 0:4, 0:512], xd8[:, 0:4, 0:512])
            nc.sync.dma_start(xs8[:, 4:8, 0:512], xd8[:, 4:8, 0:512])
            nc.sync.dma_start(hs8[:, 0:4, 0:512], hd8[:, 0:4, 0:512])
            nc.sync.dma_start(hs8[:, 4:8, 0:512], hd8[:, 4:8, 0:512])
            nc.sync.dma_start(xs8[:, :, 512:1024], xd8[:, :, 512:1024])
            nc.sync.dma_start(hs8[:, :, 512:1024], hd8[:, :, 512:1024])
            nc.sync.dma_start(xs3[:, :, 512:1024], xd3[:, :, 512:1024])

            # scalar ring: gate weights first (these become static DMAs and
            # feed the very first matmuls), then the candidate feeds, then
            # the rest of j0's consts and j1's prefetch.
            t0 = NJ * 2048
            nc.scalar.dma_start(wz0[:, 0:1024], Wg[:, 0:1024])
            nc.scalar.dma_start(wz0[:, 1024:2048], Wg[:, 1024:2048])
            nc.scalar.dma_start(wr0[:, 0:1024], Wg[:, t0 : t0 + 1024])
            nc.scalar.dma_start(wr0[:, 1024:2048], Wg[:, t0 + 1024 : t0 + 2048])
            nc.scalar.dma_start(bg_sb[:], bg[:])
            nc.scalar.dma_start(whc0[:], Whc[:, 0:H])
            nc.scalar.dma_start(xs3[:, :, 0:512], xd3[:, :, 0:512])
            nc.scalar.dma_start(wc0[:], Wc[:, 0:H])
            nc.scalar.dma_start(h16_sb[:, 0:BL], h16[:, 0:BL])
            nc.scalar.dma_start(bc_sb[:], bc[:])
            nc.scalar.dma_start(bhc_sb[:], bhc[:])
            # j1 prefetch, all on the scalar ring (sync still streams b1 acts)
            nc.scalar.dma_start(nxt[0][:], Wg[:, 1 * 2048 : 2 * 2048])
            nc.scalar.dma_start(nxt[2][:], Whc[:, 1 * H : 2 * H])
            nc.scalar.dma_start(nxt[1][:], Wg[:, (NJ + 1) * 2048 : (NJ + 2) * 2048])
            nc.scalar.dma_start(nxt[3][:], Wc[:, H : 2 * H])
            nc.scalar.dma_start(h16_sb[:, BL : 2 * BL], h16[:, BL : 2 * BL])

            # compute j0 in DMA-arrival order: both gates' x-pairs first,
            # interleaved PSUM groups; px0 before the b1 h-pairs (its feed
            # rides the scalar ring and lands before h8's b1 half).
            pz0 = ppool.tile([P, 512], F32, tag="ps")
            pr0 = ppool.tile([P, 512], F32, tag="ps")
            gate_pairs(pz0, wz03, 0, range(4), True, False)
            gate_pairs(pr0, wr03, 0, range(4), True, False)
            gate_pairs(pz0, wz03, 0, range(4, 8), False, True)
            gate_pairs(pr0, wr03, 0, range(4, 8), False, True)
            z0 = sigmoid_z(pz0, 0)
            zh0 = make_zh(z0, 0)
            r0 = sigmoid_r(pr0, 0)
            ph0 = ppool.tile([P, 512], F32, tag="ps")
            hc_dr(ph0, whc03, 0)

            pz1 = ppool.tile([P, 512], F32, tag="ps")
            pr1 = ppool.tile([P, 512], F32, tag="ps")
            gate_pairs(pz1, wz03, 512, range(4), True, False)
            gate_pairs(pr1, wr03, 512, range(4), True, False)
            px0 = ppool.tile([P, 512], F32, tag="ps")
            xc_mm(px0, wc0, 0)
            gate_pairs(pz1, wz03, 512, range(4, 8), False, True)
            gate_pairs(pr1, wr03, 512, range(4, 8), False, True)
            z1 = sigmoid_z(pz1, 0)
            zh1 = make_zh(z1, 512)
            r1 = sigmoid_r(pr1, 0)
            blend(0, 0, ph0, px0, r0, z0, zh0)

            ph1 = ppool.tile([P, 512], F32, tag="ps")
            hc_dr(ph1, whc03, 512)
            px1 = ppool.tile([P, 512], F32, tag="ps")
            xc_mm(px1, wc0, 512)
            blend(0, 512, ph1, px1, r1, z1, zh1)

            cur = nxt

            # ---------------- j = 1..7: steady state ----------------
            for j in range(1, NJ):
                if j + 1 < NJ:
                    nxt = alloc_w()
                    load_w(nxt, j + 1)
                wz, wr, whc_w, wc_w, wz3, wr3, whc3 = cur

                for b in range(NB):
                    b0 = b * 512
                    hoff = j * BL + b0
                    last = j == NJ - 1

                    pz = ppool.tile([P, 512], F32, tag="ps")
                    gate_dr(pz, wz3, b0)
                    z_sb = sigmoid_z(pz, j)
                    zh = make_zh(z_sb, hoff)
                    pr = ppool.tile([P, 512], F32, tag="ps")
                    gate_dr(pr, wr3, b0)
                    r_sb = sigmoid_r(pr, j)

                    ph = ppool.tile([P, 512], F32, tag="ps")
                    hc_dr(ph, whc3, b0)
                    px = ppool.tile([P, 512], F32, tag="ps")
                    xc_mm(px, wc_w, b0)
                    blend(j, hoff, ph, px, r_sb, z_sb, zh, halves=2 if last else 1)

                cur = nxt

    nc.compile()
    return nc


def _pack_weights(W_ih, b_ih, W_hh, b_hh, W_c, b_c, W_hc, b_hc):
    f8 = ml_dtypes.float8_e4m3
    Wg_full = np.concatenate([W_ih, W_hh], axis=0)  # [2H, 2H] = [k, o]
    WgH = np.ascontiguousarray(
        (Wg_full * WS).reshape(16, P, 16, P).transpose(1, 2, 0, 3).reshape(P, 16 * 2048)
    ).astype(f8)
    WcH = np.ascontiguousarray(
        (W_c * WS).reshape(KC, P, NJ, P).transpose(1, 2, 0, 3).reshape(P, NJ * H)
    ).astype(np.float16)
    WhcH = np.ascontiguousarray(
        (W_hc * WS).reshape(KC, P, NJ, P).transpose(1, 2, 0, 3).reshape(P, NJ * H)
    ).astype(f8)
    bgH = np.ascontiguousarray((b_ih + b_hh).reshape(16, P).T).astype(np.float32)
    bcH = np.ascontiguousarray(b_c.reshape(NJ, P).T).astype(np.float32)
    bhcH = np.ascontiguousarray((b_hc * WS).reshape(NJ, P).T).astype(np.float32)
    return WgH, WcH, WhcH, bgH, bcH, bhcH


def _pack_acts(a, dtype):
    # [BL, H] -> [p, kc*BL + b] with a[b, kc*128+p]
    return np.ascontiguousarray(
        a.T.reshape(KC, P, BL).transpose(1, 0, 2).reshape(P, KC * BL)
    ).astype(dtype)


def _make_in_maps(input, hx, W_ih, b_ih, W_hh, b_hh, W_c, b_c, W_hc, b_hc):
    input = np.asarray(input, np.float32)
    hx = np.asarray(hx, np.float32)
    WgH, WcH, WhcH, bgH, bcH, bhcH = _pack_weights(
        np.asarray(W_ih, np.float32), np.asarray(b_ih, np.float32),
        np.asarray(W_hh, np.float32), np.asarray(b_hh, np.float32),
        np.asarray(W_c, np.float32), np.asarray(b_c, np.float32),
        np.asarray(W_hc, np.float32), np.asarray(b_hc, np.float32),
    )
    f8 = ml_dtypes.float8_e4m3
    e3 = ml_dtypes.float8_e3m4
    in_maps = []
    for i in range(N_CORES):
        xs = input[i * BL : (i + 1) * BL]
        hs = hx[i * BL : (i + 1) * BL]
        in_maps.append(
            {
                "x8": _pack_acts(xs, f8),
                "h8": _pack_acts(hs, f8),
                "xe3": _pack_acts(xs, e3),
                "h16": _pack_acts(hs, np.float16),
                "Wg": WgH,
                "Wc": WcH,
                "Whc": WhcH,
                "bg": bgH,
                "bc": bcH,
                "bhc": bhcH,
            }
        )
    return in_maps


def kernel(input, hx, W_ih, b_ih, W_hh, b_hh, W_c, b_c, W_hc, b_hc):
    if "nc" not in _CACHE:
        _CACHE["nc"] = _build_program()
    nc = _CACHE["nc"]

    in_maps = _make_in_maps(
        input, hx, W_ih, b_ih, W_hh, b_hh, W_c, b_c, W_hc, b_hc
    )

    res = run_bass_kernel_spmd(nc, in_maps, core_ids=list(range(N_CORES)))
    out = np.empty((B, H), np.float32)
    for i, r in enumerate(res.results):
        o = np.asarray(r["outT"], np.float32).reshape(P, NJ, BL).transpose(2, 1, 0).reshape(BL, H)
        out[i * BL : (i + 1) * BL] = o
    return out
